# revision 11
# baseline (speedup 1.0000x reference)
"""Grouped linear (MoE expert GEMM) Trainium2 Bass kernel.

Problem: x [16384, 2048] f32 expert-sorted tokens, weight [8, 2048, 2048]
(E, out, in), bias [8, 2048]. out[t] = x[t] @ W[e(t)].T + bias[e(t)].

Sharding: expert-parallel across 8 NeuronCores. Tokens are expert-sorted and
counts sum to T, so the "all-to-all" is a host-side contiguous slice per
expert. Each core runs an identical (SPMD) dense GEMM over its expert's
tokens padded to capacity (2304 rows = 18 tiles of 128).

Device kernel (per core):
  out[2304, 2048] = X[2304, 2048] @ W.T[2048, 2048] + bias
  - W^T held fully resident in SBUF (16 k-tiles x [128, 2048] = 128KB/part)
  - per 128-token m-tile: accumulate 4 PSUM banks [128, 512] over 16 k-tiles
    with fp32r (FP22-multiply, fp32-accumulate) matmuls at 1 cycle/row
  - bias add fused into the PSUM->SBUF eviction on the vector engine
  - all DMAs are contiguous-8KB-per-partition 1MB transfers (host pre-swizzles
    x into the exact SBUF layout; weights pre-transposed to k-major on host)
"""

import numpy as np

E = 8
IN = 2048
OUT = 2048
CAP = 2304
P = 128
MT = CAP // P      # 18 m-tiles (token tiles)
KT = IN // P       # 16 k-tiles (contraction)
NF = 512           # matmul free dim / PSUM bank (fp32)
NB = OUT // NF     # 4 n-blocks

_NC_CACHE = {}


def _build_nc(mt=MT, kt=KT, nb=NB, x_bufs=3, o_bufs=3, ps_bufs=2):
    import concourse.mybir as mybir
    import concourse.tile as tile
    from concourse import bacc

    f32 = mybir.dt.float32
    f32r = mybir.dt.float32r
    n_in = kt * P
    n_out = nb * NF

    nc = bacc.Bacc("TRN2", target_bir_lowering=False, debug=False)
    xs = nc.dram_tensor("xs", [mt, P, n_in], f32r, kind="ExternalInput")
    ws = nc.dram_tensor("ws", [kt, P, n_out], f32r, kind="ExternalInput")
    bs = nc.dram_tensor("bs", [P, n_out], f32, kind="ExternalInput")
    out = nc.dram_tensor("out", [mt * P, n_out], f32, kind="ExternalOutput")

    with tile.TileContext(nc) as tc:
        with (
            tc.tile_pool(name="wpool", bufs=1) as wpool,
            tc.tile_pool(name="xpool", bufs=x_bufs) as xpool,
            tc.tile_pool(name="bpool", bufs=1) as bpool,
            tc.tile_pool(name="opool", bufs=o_bufs) as opool,
            tc.tile_pool(name="pspool", bufs=ps_bufs, space="PSUM") as pspool,
        ):
            bias_sb = bpool.tile([P, n_out], f32, name="bias")
            nc.sync.dma_start(bias_sb[:], bs[:])

            w_tiles = []
            for k in range(kt):
                wt = wpool.tile([P, n_out], f32r, name=f"w{k}", tag=f"w{k}")
                nc.sync.dma_start(wt[:], ws[k])
                w_tiles.append(wt)

            for m in range(mt):
                x_t = xpool.tile([P, n_in], f32r, name="xt", tag="xt")
                nc.scalar.dma_start(x_t[:], xs[m])

                psums = [
                    pspool.tile([P, NF], f32, name=f"ps{n}", tag=f"ps{n}")
                    for n in range(nb)
                ]
                for k in range(kt):
                    lhsT = x_t[:, k * P : (k + 1) * P]
                    for n in range(nb):
                        nc.tensor.matmul(
                            psums[n][:],
                            lhsT,
                            w_tiles[k][:, n * NF : (n + 1) * NF],
                            start=(k == 0),
                            stop=(k == kt - 1),
                        )

                o_t = opool.tile([P, n_out], f32, name="ot", tag="ot")
                for n in range(nb):
                    nc.vector.tensor_add(
                        o_t[:, n * NF : (n + 1) * NF],
                        psums[n][:],
                        bias_sb[:, n * NF : (n + 1) * NF],
                    )
                nc.gpsimd.dma_start(out[m * P : (m + 1) * P, :], o_t[:])

    nc.compile()
    return nc


def _get_nc():
    if "nc" not in _NC_CACHE:
        _NC_CACHE["nc"] = _build_nc()
    return _NC_CACHE["nc"]


def _shard_inputs(x, weight, bias, counts):
    """Per-core input maps: slice+pad tokens, swizzle to SBUF layouts."""
    offs = np.zeros(E + 1, np.int64)
    offs[1:] = np.cumsum(counts)
    in_maps = []
    for e in range(E):
        cnt = int(counts[e])
        off = int(offs[e])
        xe = np.zeros((CAP, IN), np.float32)
        xe[:cnt] = x[off : off + cnt]
        # xs[m, p, k*128+j] = xe[m*128+j, k*128+p]  (k on partitions)
        xsb = np.ascontiguousarray(
            xe.reshape(MT, P, KT, P).transpose(0, 3, 2, 1)
        ).reshape(MT, P, IN)
        # ws[k, p, o] = W[o, k*128+p]  (k on partitions)
        wsb = np.ascontiguousarray(weight[e].T).reshape(KT, P, OUT)
        bsb = np.ascontiguousarray(np.broadcast_to(bias[e], (P, OUT)))
        in_maps.append({"xs": xsb, "ws": wsb, "bs": bsb})
    return in_maps, offs


def kernel(x, weight, bias, tokens_per_expert, capacity=None):
    from concourse.bass_utils import run_bass_kernel_spmd

    x = np.ascontiguousarray(np.asarray(x), dtype=np.float32)
    weight = np.asarray(weight, dtype=np.float32)
    bias = np.asarray(bias, dtype=np.float32)
    counts = np.asarray(tokens_per_expert).astype(np.int64)
    assert x.shape[1] == IN and weight.shape == (E, OUT, IN)
    assert counts.sum() == x.shape[0]
    assert counts.max() <= CAP

    in_maps, offs = _shard_inputs(x, weight, bias, counts)
    nc = _get_nc()
    res = run_bass_kernel_spmd(nc, in_maps, core_ids=list(range(E)))

    out = np.empty((x.shape[0], OUT), np.float32)
    for e in range(E):
        cnt = int(counts[e])
        off = int(offs[e])
        out[off : off + cnt] = res.results[e]["out"][:cnt]
    return out


# revision 16
# speedup vs baseline: 347.3942x; 347.3942x over previous
"""Grouped linear (MoE expert GEMM) Trainium2 Bass kernel.

Problem: x [16384, 2048] f32 expert-sorted tokens, weight [8, 2048, 2048]
(E, out, in), bias [8, 2048]. out[t] = x[t] @ W[e(t)].T + bias[e(t)].

Sharding: expert-parallel across 8 NeuronCores. Tokens are expert-sorted and
counts sum to T, so the "all-to-all" is a host-side contiguous slice per
expert. Each core runs an identical (SPMD) dense GEMM over its expert's
tokens padded to capacity (2304 rows = 18 tiles of 128).

Device kernel (per core):
  out[2304, 2048] = X[2304, 2048] @ W.T[2048, 2048] + bias
  - W^T held fully resident in SBUF (16 k-tiles x [128, 2048] = 128KB/part)
  - per 128-token m-tile: accumulate 4 PSUM banks [128, 512] over 16 k-tiles
    with fp32r (FP22-multiply, fp32-accumulate) matmuls at 1 cycle/row
  - bias add fused into the PSUM->SBUF eviction on the vector engine
  - all DMAs are contiguous-8KB-per-partition 1MB transfers (host pre-swizzles
    x into the exact SBUF layout; weights pre-transposed to k-major on host)
"""

import numpy as np

E = 8
IN = 2048
OUT = 2048
CAP = 2304
P = 128
MT = CAP // P      # 18 m-tiles (token tiles)
KT = IN // P       # 16 k-tiles (contraction)
NF = 512           # matmul free dim / PSUM bank (fp32)
NB = OUT // NF     # 4 n-blocks

_NC_CACHE = {}


def _build_nc(
    mt=MT, kt=KT, nb=NB, x_bufs=3, o_bufs=3, ps_bufs=2, reps=1,
    inline=None, internal_out=False,
):
    """inline / internal_out are for timing experiments only: inline bakes the
    input data into the NEFF as Const tensors and internal_out keeps the big
    output on-device, so per-call host<->device marshaling is ~zero."""
    import concourse.bass as bass
    import concourse.mybir as mybir
    import concourse.tile as tile
    from concourse import bacc

    f32 = mybir.dt.float32
    f32r = mybir.dt.float32r
    n_in = kt * P
    n_out = nb * NF

    nc = bacc.Bacc("TRN2", target_bir_lowering=False, debug=False)

    def _mk_in(name, shape, dtype):
        if inline is None:
            return nc.dram_tensor(name, shape, dtype, kind="ExternalInput")
        h = nc.inline_tensor(inline[name], name=name)
        if h.dtype != dtype:
            mls = nc.lookup_mls(name)
            mls.dtype = dtype
            for ml in mls.memorylocations or []:
                ml.dtype = dtype
            h = bass.DRamTensorHandle(name, list(shape), dtype)
        return h

    xs = _mk_in("xs", [mt, P, n_in], f32r)
    ws = _mk_in("ws", [kt, P, n_out], f32r)
    bs = _mk_in("bs", [P, n_out], f32)
    out = nc.dram_tensor(
        "out", [mt * P, n_out], f32,
        kind="Internal" if internal_out else "ExternalOutput",
    )
    flag = (
        nc.dram_tensor("flag", [1, 1], f32, kind="ExternalOutput")
        if internal_out
        else None
    )

    with tile.TileContext(nc) as tc:
        with (
            tc.tile_pool(name="wpool", bufs=1) as wpool,
            tc.tile_pool(name="xpool", bufs=x_bufs) as xpool,
            tc.tile_pool(name="bpool", bufs=1) as bpool,
            tc.tile_pool(name="opool", bufs=o_bufs) as opool,
            tc.tile_pool(name="pspool", bufs=ps_bufs, space="PSUM") as pspool,
        ):
          for _rep in range(reps):
            bias_sb = bpool.tile([P, n_out], f32, name="bias")
            nc.sync.dma_start(bias_sb[:], bs[:])

            w_tiles = []
            for k in range(kt):
                wt = wpool.tile([P, n_out], f32r, name=f"w{k}", tag=f"w{k}")
                nc.sync.dma_start(wt[:], ws[k])
                w_tiles.append(wt)

            for m in range(mt):
                x_t = xpool.tile([P, n_in], f32r, name="xt", tag="xt")
                nc.scalar.dma_start(x_t[:], xs[m])

                psums = [
                    pspool.tile([P, NF], f32, name=f"ps{n}", tag=f"ps{n}")
                    for n in range(nb)
                ]
                for k in range(kt):
                    lhsT = x_t[:, k * P : (k + 1) * P]
                    for n in range(nb):
                        nc.tensor.matmul(
                            psums[n][:],
                            lhsT,
                            w_tiles[k][:, n * NF : (n + 1) * NF],
                            start=(k == 0),
                            stop=(k == kt - 1),
                        )

                o_t = opool.tile([P, n_out], f32, name="ot", tag="ot")
                for n in range(nb):
                    nc.vector.tensor_add(
                        o_t[:, n * NF : (n + 1) * NF],
                        psums[n][:],
                        bias_sb[:, n * NF : (n + 1) * NF],
                    )
                nc.gpsimd.dma_start(out[m * P : (m + 1) * P, :], o_t[:])
                if flag is not None and m == mt - 1 and _rep == reps - 1:
                    nc.gpsimd.dma_start(flag[:], o_t[0:1, 0:1])

    nc.compile()
    return nc


def _get_nc():
    if "nc" not in _NC_CACHE:
        _NC_CACHE["nc"] = _build_nc()
    return _NC_CACHE["nc"]


def _shard_inputs(x, weight, bias, counts):
    """Per-core input maps: slice+pad tokens, swizzle to SBUF layouts."""
    offs = np.zeros(E + 1, np.int64)
    offs[1:] = np.cumsum(counts)
    in_maps = []
    for e in range(E):
        cnt = int(counts[e])
        off = int(offs[e])
        xe = np.zeros((CAP, IN), np.float32)
        xe[:cnt] = x[off : off + cnt]
        # xs[m, p, k*128+j] = xe[m*128+j, k*128+p]  (k on partitions)
        xsb = np.ascontiguousarray(
            xe.reshape(MT, P, KT, P).transpose(0, 3, 2, 1)
        ).reshape(MT, P, IN)
        # ws[k, p, o] = W[o, k*128+p]  (k on partitions)
        wsb = np.ascontiguousarray(weight[e].T).reshape(KT, P, OUT)
        bsb = np.ascontiguousarray(np.broadcast_to(bias[e], (P, OUT)))
        in_maps.append({"xs": xsb, "ws": wsb, "bs": bsb})
    return in_maps, offs


def kernel(x, weight, bias, tokens_per_expert, capacity=None):
    from concourse.bass_utils import run_bass_kernel_spmd

    x = np.ascontiguousarray(np.asarray(x), dtype=np.float32)
    weight = np.asarray(weight, dtype=np.float32)
    bias = np.asarray(bias, dtype=np.float32)
    counts = np.asarray(tokens_per_expert).astype(np.int64)
    assert x.shape[1] == IN and weight.shape == (E, OUT, IN)
    assert counts.sum() == x.shape[0]
    assert counts.max() <= CAP

    in_maps, offs = _shard_inputs(x, weight, bias, counts)
    nc = _get_nc()
    # the axon-tunneled device occasionally reports a transient
    # NRT_EXEC_UNIT_UNRECOVERABLE on the first execute; a retry clears it
    last_err = None
    for _attempt in range(3):
        try:
            res = run_bass_kernel_spmd(nc, in_maps, core_ids=list(range(E)))
            break
        except Exception as err:  # noqa: BLE001
            last_err = err
    else:
        raise last_err

    out = np.empty((x.shape[0], OUT), np.float32)
    for e in range(E):
        cnt = int(counts[e])
        off = int(offs[e])
        out[off : off + cnt] = res.results[e]["out"][:cnt]
    return out
